# revision 5
# baseline (speedup 1.0000x reference)
"""BitLinear inference kernel for Trainium2: y = (x @ W_q^T) * s + bias.

Shapes: x [8192, 4096] f32, w_q [4096, 4096] ternary {-1,0,1}, s [1] f32,
bias [4096] f32 -> y [8192, 4096] f32.

Strategy:
- Data-parallel across 8 NeuronCores: each core computes 1024 of the 8192
  token rows against the full weight matrix.
- Mixed-precision contraction over D_IN: 14 pairs of 128-wide k-tiles run
  as fp8e4m3 matmuls in DoubleRow perf mode (2 MACs/cell/cycle, ~1.77x
  the bf16 PE rate), the remaining 4 k-tiles run in bf16. All accumulate
  into the same fp32 PSUM group.
- The ternary weights are exact in fp8/bf16; only x quantizes. To share
  one PSUM scale between the fp8 and bf16 parts, fp8 weights are W/32
  (+-2^-5, exact in e4m3) and fp8 x is e4m3(x*s*32). The per-tensor scale
  s is folded into x on the host. Measured rel err 1.933e-2 (gate 2e-2);
  error comes almost entirely from e4m3(x) on 28 of 32 k-tiles
  (~2.48e-2 * sqrt(28/32)).
- Output computed transposed (y^T tiles [n=128, m=512]) so per-channel
  bias lands on partitions; one ScalarE/VectorE op per tile does
  PSUM->SBUF eviction + bias add.
- W streams through SBUF once per pass (fp8 halves the W traffic); x^T
  stays resident in SBUF. y writeback rides the Activation HWDGE queue,
  W stream the SP queue.
"""

import numpy as np
import ml_dtypes

M_TOTAL = 8192
D_IN = 4096
D_OUT = 4096
N_CORES = 8
P = 128
M = M_TOTAL // N_CORES  # 1024 tokens per core
NT = D_OUT // P         # 32 output-channel tiles
MF = 512                # matmul moving free dim (= one fp32 PSUM bank)
MC = M // MF            # 2 m-chunks per core
KB = 4                  # bf16 k-tiles (trailing)
KP = 14                 # fp8 DoubleRow k-tile pairs (leading 28 k-tiles)
QEXP = 5                # fp8 scale: x*2^QEXP, w*2^-QEXP

_CACHE = {}


def build_nc(repeats=1, psum_bufs=2, w_bufs=3, o_bufs=4, w_split=1,
             warmup_mms=0, evict="mixed", out_queue="scalar"):
    """Build + compile the per-core Bass module.

    repeats > 1 wraps the compute in a hardware loop that recomputes the
    (identical) output that many times -- used only for timing runs.
    """
    import concourse.mybir as mybir
    import concourse.tile as tile
    from concourse import bacc

    nc = bacc.Bacc(
        "TRN2",
        target_bir_lowering=False,
        debug=False,
        num_devices=N_CORES,
    )
    bf16 = mybir.dt.bfloat16
    f8 = mybir.dt.float8e4
    f32 = mybir.dt.float32
    DR = mybir.MatmulPerfMode.DoubleRow

    xb = nc.dram_tensor("xb", [KB * P, M], bf16, kind="ExternalInput")
    wb = nc.dram_tensor("wb", [NT, P, KB, P], bf16, kind="ExternalInput")
    xf = nc.dram_tensor("xf", [KP, P, 2, M], f8, kind="ExternalInput")
    wf = nc.dram_tensor("wf", [NT, P, KP, 2, P], f8, kind="ExternalInput")
    bt = nc.dram_tensor("bt", [P, NT], f32, kind="ExternalInput")
    yt = nc.dram_tensor("yt", [D_OUT, M], f32, kind="ExternalOutput")

    xb_r = xb.ap().rearrange("(kb p) m -> p kb m", p=P)
    wb_a = wb.ap()
    xf_a = xf.ap()
    wf_a = wf.ap()
    yt_r = yt.ap().rearrange("(nt p) m -> p nt m", p=P)
    ident = mybir.ActivationFunctionType.Identity

    with tile.TileContext(nc) as tc:
        with (
            tc.tile_pool(name="xpool", bufs=1) as xpool,
            tc.tile_pool(name="cpool", bufs=1) as cpool,
            tc.tile_pool(name="wpool", bufs=w_bufs) as wpool,
            tc.tile_pool(name="opool", bufs=o_bufs) as opool,
            tc.tile_pool(name="pspool", bufs=psum_bufs, space="PSUM") as pspool,
        ):
            bias_sb = cpool.tile([P, NT], f32, tag="bias")
            nc.sync.dma_start(bias_sb[:], bt.ap())

            # x preload: resident in SBUF for the whole run, spread across
            # both HWDGE queues.
            xs_bf = []
            for kb in range(KB):
                xk = xpool.tile([P, M], bf16, tag=f"xb{kb}")
                eng = nc.sync if kb % 2 else nc.scalar
                eng.dma_start(xk[:], xb_r[:, kb, :])
                xs_bf.append(xk)
            xs_f8 = []
            for kp in range(KP):
                xk = xpool.tile([P, 2, M], f8, tag=f"xf{kp}")
                eng = nc.sync if kp % 2 else nc.scalar
                eng.dma_start(xk[:], xf_a[kp, :, :, :])
                xs_f8.append(xk)

            if warmup_mms:
                # Ramp the PE clock during the x preload with junk matmuls
                # into a scratch PSUM bank that is never read.
                wu = cpool.tile([P, MF], bf16, tag="warmup")
                nc.vector.memset(wu[:], 0.0)
                wu_ps = pspool.tile([P, MF], f32, tag="wups", name="wups",
                                    bufs=1)
                for _ in range(warmup_mms):
                    nc.tensor.matmul(
                        wu_ps[:], wu[:, :P], wu[:], start=True, stop=True,
                    )

            def load_w(nt):
                w_f8 = wpool.tile([P, KP, 2, P], f8, tag="wf8",
                                  name=f"wf8_{nt}")
                ck = max(1, KP // max(1, w_split // 2))
                for c0 in range(0, KP, ck):
                    sl = slice(c0, min(c0 + ck, KP))
                    nc.sync.dma_start(w_f8[:, sl, :, :], wf_a[nt, :, sl, :, :])
                w_bf = wpool.tile([P, KB, P], bf16, tag="wbf",
                                  name=f"wbf_{nt}")
                nc.sync.dma_start(w_bf[:], wb_a[nt, :, :, :])
                return w_f8, w_bf

            def epilogue(nt, pss):
                for mc in range(MC):
                    o_sb = opool.tile([P, MF], f32, tag=f"o{mc}",
                                      name=f"o{nt}_{mc}")
                    bias_col = bias_sb[:, nt:nt + 1]
                    if evict == "mixed" and mc == 0:
                        nc.scalar.activation(
                            o_sb[:], pss[mc][:], ident,
                            bias=bias_col, scale=1.0,
                        )
                    else:
                        nc.vector.tensor_scalar_add(
                            o_sb[:], pss[mc][:], bias_col,
                        )
                    out_eng = nc.sync if out_queue == "sync" else nc.scalar
                    out_eng.dma_start(
                        yt_r[:, nt, mc * MF:(mc + 1) * MF], o_sb[:],
                    )

            def body(_iv=None):
                for nt in range(NT):
                    w_f8, w_bf = load_w(nt)
                    pss = [
                        pspool.tile([P, MF], f32, tag=f"ps{mc}",
                                    name=f"ps{mc}")
                        for mc in range(MC)
                    ]
                    n_steps = KP + KB
                    for kp in range(KP):
                        lhsT = w_f8[:, kp, :, :]
                        for mc in range(MC):
                            nc.tensor.matmul(
                                pss[mc][:], lhsT,
                                xs_f8[kp][:, :, mc * MF:(mc + 1) * MF],
                                start=(kp == 0),
                                stop=(KB == 0 and kp == KP - 1),
                                perf_mode=DR,
                            )
                    for kb in range(KB):
                        lhsT = w_bf[:, kb, :]
                        for mc in range(MC):
                            nc.tensor.matmul(
                                pss[mc][:], lhsT,
                                xs_bf[kb][:, mc * MF:(mc + 1) * MF],
                                start=(KP == 0 and kb == 0),
                                stop=(kb == KB - 1),
                            )
                    epilogue(nt, pss)

            if repeats == 1:
                body()
            else:
                with tc.For_i(0, repeats, 1) as iv:
                    body(iv)

    nc.compile()
    return nc


def prep_inputs(x, w_q, s, bias):
    bf16 = ml_dtypes.bfloat16
    f8 = ml_dtypes.float8_e4m3
    # Convert up front: the harness may pass jax arrays, and slicing those
    # would dispatch device ops.
    x = np.asarray(x)
    s_val = np.float32(np.asarray(s).reshape(-1)[0])
    w = np.asarray(w_q).astype(np.float32)
    KF = 2 * KP  # leading 2*KP k-tiles in fp8, trailing KB in bf16
    DF = KF * P

    # wf[nt, p, kp, i, m] = W[nt*128+m, (2kp+i)*128+p] / 2^QEXP
    wt4 = w.reshape(NT, P, D_IN // P, P).transpose(0, 3, 2, 1)  # [nt,p,ko,m]
    wf = np.ascontiguousarray(
        wt4[:, :, :KF, :].reshape(NT, P, KP, 2, P) * np.float32(2.0 ** -QEXP)
    ).astype(f8)
    wb = np.ascontiguousarray(wt4[:, :, KF:, :]).astype(bf16)
    bt = np.ascontiguousarray(
        np.asarray(bias, dtype=np.float32).reshape(NT, P).T
    )

    in_maps = []
    for c in range(N_CORES):
        xc = np.asarray(x[c * M:(c + 1) * M], dtype=np.float32) * s_val
        xtc = xc.T  # [D_IN, M] f32
        xf = np.ascontiguousarray(
            (xtc[:DF] * np.float32(2.0 ** QEXP))
            .reshape(KP, 2, P, M).transpose(0, 2, 1, 3)
        ).astype(f8)  # [KP, P, 2, M]
        xb = np.ascontiguousarray(xtc[DF:]).astype(bf16)
        in_maps.append({"xf": xf, "wf": wf, "xb": xb, "wb": wb, "bt": bt})
    return in_maps


def run(nc, in_maps, **kwargs):
    from concourse import bass_utils

    return bass_utils.run_bass_kernel_spmd(
        nc, in_maps, core_ids=list(range(N_CORES)), **kwargs
    )


def kernel(x, w_q, s, bias):
    nc = _CACHE.get("nc")
    if nc is None:
        nc = _CACHE["nc"] = build_nc()
    in_maps = prep_inputs(x, w_q, s, bias)
    res = run(nc, in_maps)
    y = np.empty((M_TOTAL, D_OUT), dtype=np.float32)
    for c in range(N_CORES):
        y[c * M:(c + 1) * M] = res.results[c]["yt"].T
    return y


# revision 6
# speedup vs baseline: 1.0262x; 1.0262x over previous
"""BitLinear inference kernel for Trainium2: y = (x @ W_q^T) * s + bias.

Shapes: x [8192, 4096] f32, w_q [4096, 4096] ternary {-1,0,1}, s [1] f32,
bias [4096] f32 -> y [8192, 4096] f32.

Strategy:
- Data-parallel across 8 NeuronCores: each core computes 1024 of the 8192
  token rows against the full weight matrix.
- Mixed-precision contraction over D_IN: 14 pairs of 128-wide k-tiles run
  as fp8e4m3 matmuls in DoubleRow perf mode (2 MACs/cell/cycle, ~1.77x
  the bf16 PE rate), the remaining 4 k-tiles run in bf16. All accumulate
  into the same fp32 PSUM group.
- The ternary weights are exact in fp8/bf16; only x quantizes. To share
  one PSUM scale between the fp8 and bf16 parts, fp8 weights are W/32
  (+-2^-5, exact in e4m3) and fp8 x is e4m3(x*s*32). The per-tensor scale
  s is folded into x on the host. Measured rel err 1.933e-2 (gate 2e-2);
  error comes almost entirely from e4m3(x) on 28 of 32 k-tiles
  (~2.48e-2 * sqrt(28/32)).
- Output computed transposed (y^T tiles [n=128, m=512]) so per-channel
  bias lands on partitions; one ScalarE/VectorE op per tile does
  PSUM->SBUF eviction + bias add, writing bf16 (halves eviction writes
  and output DMA; the host upcasts to f32 — adds only 7e-5 rel err in
  quadrature, measured 1.940e-2 total).
- W streams through SBUF once per pass (fp8 halves the W traffic); x^T
  stays resident in SBUF. y writeback rides the Activation HWDGE queue,
  W stream the SP queue.
"""

import numpy as np
import ml_dtypes

M_TOTAL = 8192
D_IN = 4096
D_OUT = 4096
N_CORES = 8
P = 128
M = M_TOTAL // N_CORES  # 1024 tokens per core
NT = D_OUT // P         # 32 output-channel tiles
MF = 512                # matmul moving free dim (= one fp32 PSUM bank)
MC = M // MF            # 2 m-chunks per core
KB = 4                  # bf16 k-tiles (trailing)
KP = 14                 # fp8 DoubleRow k-tile pairs (leading 28 k-tiles)
QEXP = 5                # fp8 scale: x*2^QEXP, w*2^-QEXP

_CACHE = {}


def build_nc(repeats=1, psum_bufs=2, w_bufs=3, o_bufs=4, w_split=1,
             warmup_mms=0, evict="mixed", out_queue="scalar"):
    """Build + compile the per-core Bass module.

    repeats > 1 wraps the compute in a hardware loop that recomputes the
    (identical) output that many times -- used only for timing runs.
    """
    import concourse.mybir as mybir
    import concourse.tile as tile
    from concourse import bacc

    nc = bacc.Bacc(
        "TRN2",
        target_bir_lowering=False,
        debug=False,
        num_devices=N_CORES,
    )
    bf16 = mybir.dt.bfloat16
    f8 = mybir.dt.float8e4
    f32 = mybir.dt.float32
    DR = mybir.MatmulPerfMode.DoubleRow

    xb = nc.dram_tensor("xb", [KB * P, M], bf16, kind="ExternalInput")
    wb = nc.dram_tensor("wb", [NT, P, KB, P], bf16, kind="ExternalInput")
    xf = nc.dram_tensor("xf", [KP, P, 2, M], f8, kind="ExternalInput")
    wf = nc.dram_tensor("wf", [NT, P, KP, 2, P], f8, kind="ExternalInput")
    bt = nc.dram_tensor("bt", [P, NT], f32, kind="ExternalInput")
    yt = nc.dram_tensor("yt", [D_OUT, M], bf16, kind="ExternalOutput")

    xb_r = xb.ap().rearrange("(kb p) m -> p kb m", p=P)
    wb_a = wb.ap()
    xf_a = xf.ap()
    wf_a = wf.ap()
    yt_r = yt.ap().rearrange("(nt p) m -> p nt m", p=P)
    ident = mybir.ActivationFunctionType.Identity

    with tile.TileContext(nc) as tc:
        with (
            tc.tile_pool(name="xpool", bufs=1) as xpool,
            tc.tile_pool(name="cpool", bufs=1) as cpool,
            tc.tile_pool(name="wpool", bufs=w_bufs) as wpool,
            tc.tile_pool(name="opool", bufs=o_bufs) as opool,
            tc.tile_pool(name="pspool", bufs=psum_bufs, space="PSUM") as pspool,
        ):
            bias_sb = cpool.tile([P, NT], f32, tag="bias")
            nc.sync.dma_start(bias_sb[:], bt.ap())

            # x preload: resident in SBUF for the whole run, spread across
            # both HWDGE queues.
            xs_bf = []
            for kb in range(KB):
                xk = xpool.tile([P, M], bf16, tag=f"xb{kb}")
                eng = nc.sync if kb % 2 else nc.scalar
                eng.dma_start(xk[:], xb_r[:, kb, :])
                xs_bf.append(xk)
            xs_f8 = []
            for kp in range(KP):
                xk = xpool.tile([P, 2, M], f8, tag=f"xf{kp}")
                eng = nc.sync if kp % 2 else nc.scalar
                eng.dma_start(xk[:], xf_a[kp, :, :, :])
                xs_f8.append(xk)

            if warmup_mms:
                # Ramp the PE clock during the x preload with junk matmuls
                # into a scratch PSUM bank that is never read.
                wu = cpool.tile([P, MF], bf16, tag="warmup")
                nc.vector.memset(wu[:], 0.0)
                wu_ps = pspool.tile([P, MF], f32, tag="wups", name="wups",
                                    bufs=1)
                for _ in range(warmup_mms):
                    nc.tensor.matmul(
                        wu_ps[:], wu[:, :P], wu[:], start=True, stop=True,
                    )

            def load_w(nt):
                w_f8 = wpool.tile([P, KP, 2, P], f8, tag="wf8",
                                  name=f"wf8_{nt}")
                ck = max(1, KP // max(1, w_split // 2))
                for c0 in range(0, KP, ck):
                    sl = slice(c0, min(c0 + ck, KP))
                    nc.sync.dma_start(w_f8[:, sl, :, :], wf_a[nt, :, sl, :, :])
                w_bf = wpool.tile([P, KB, P], bf16, tag="wbf",
                                  name=f"wbf_{nt}")
                nc.sync.dma_start(w_bf[:], wb_a[nt, :, :, :])
                return w_f8, w_bf

            def epilogue(nt, pss):
                for mc in range(MC):
                    o_sb = opool.tile([P, MF], bf16, tag=f"o{mc}",
                                      name=f"o{nt}_{mc}")
                    bias_col = bias_sb[:, nt:nt + 1]
                    if evict == "mixed" and mc == 0:
                        nc.scalar.activation(
                            o_sb[:], pss[mc][:], ident,
                            bias=bias_col, scale=1.0,
                        )
                    else:
                        nc.vector.tensor_scalar_add(
                            o_sb[:], pss[mc][:], bias_col,
                        )
                    out_eng = nc.sync if out_queue == "sync" else nc.scalar
                    out_eng.dma_start(
                        yt_r[:, nt, mc * MF:(mc + 1) * MF], o_sb[:],
                    )

            def body(_iv=None):
                for nt in range(NT):
                    w_f8, w_bf = load_w(nt)
                    pss = [
                        pspool.tile([P, MF], f32, tag=f"ps{mc}",
                                    name=f"ps{mc}")
                        for mc in range(MC)
                    ]
                    n_steps = KP + KB
                    for kp in range(KP):
                        lhsT = w_f8[:, kp, :, :]
                        for mc in range(MC):
                            nc.tensor.matmul(
                                pss[mc][:], lhsT,
                                xs_f8[kp][:, :, mc * MF:(mc + 1) * MF],
                                start=(kp == 0),
                                stop=(KB == 0 and kp == KP - 1),
                                perf_mode=DR,
                            )
                    for kb in range(KB):
                        lhsT = w_bf[:, kb, :]
                        for mc in range(MC):
                            nc.tensor.matmul(
                                pss[mc][:], lhsT,
                                xs_bf[kb][:, mc * MF:(mc + 1) * MF],
                                start=(KP == 0 and kb == 0),
                                stop=(kb == KB - 1),
                            )
                    epilogue(nt, pss)

            if repeats == 1:
                body()
            else:
                with tc.For_i(0, repeats, 1) as iv:
                    body(iv)

    nc.compile()
    return nc


def prep_inputs(x, w_q, s, bias):
    bf16 = ml_dtypes.bfloat16
    f8 = ml_dtypes.float8_e4m3
    # Convert up front: the harness may pass jax arrays, and slicing those
    # would dispatch device ops.
    x = np.asarray(x)
    s_val = np.float32(np.asarray(s).reshape(-1)[0])
    w = np.asarray(w_q).astype(np.float32)
    KF = 2 * KP  # leading 2*KP k-tiles in fp8, trailing KB in bf16
    DF = KF * P

    # wf[nt, p, kp, i, m] = W[nt*128+m, (2kp+i)*128+p] / 2^QEXP
    wt4 = w.reshape(NT, P, D_IN // P, P).transpose(0, 3, 2, 1)  # [nt,p,ko,m]
    wf = np.ascontiguousarray(
        wt4[:, :, :KF, :].reshape(NT, P, KP, 2, P) * np.float32(2.0 ** -QEXP)
    ).astype(f8)
    wb = np.ascontiguousarray(wt4[:, :, KF:, :]).astype(bf16)
    bt = np.ascontiguousarray(
        np.asarray(bias, dtype=np.float32).reshape(NT, P).T
    )

    in_maps = []
    for c in range(N_CORES):
        xc = np.asarray(x[c * M:(c + 1) * M], dtype=np.float32) * s_val
        xtc = xc.T  # [D_IN, M] f32
        xf = np.ascontiguousarray(
            (xtc[:DF] * np.float32(2.0 ** QEXP))
            .reshape(KP, 2, P, M).transpose(0, 2, 1, 3)
        ).astype(f8)  # [KP, P, 2, M]
        xb = np.ascontiguousarray(xtc[DF:]).astype(bf16)
        in_maps.append({"xf": xf, "wf": wf, "xb": xb, "wb": wb, "bt": bt})
    return in_maps


def run(nc, in_maps, **kwargs):
    from concourse import bass_utils

    return bass_utils.run_bass_kernel_spmd(
        nc, in_maps, core_ids=list(range(N_CORES)), **kwargs
    )


def kernel(x, w_q, s, bias):
    nc = _CACHE.get("nc")
    if nc is None:
        nc = _CACHE["nc"] = build_nc()
    in_maps = prep_inputs(x, w_q, s, bias)
    res = run(nc, in_maps)
    y = np.empty((M_TOTAL, D_OUT), dtype=np.float32)
    for c in range(N_CORES):
        y[c * M:(c + 1) * M] = res.results[c]["yt"].T
    return y
